# revision 69
# baseline (speedup 1.0000x reference)
"""Trainium2 Bass kernel for ButterflyGlobalLinear:

    y = x @ (mask * weight)^T + bias

x: [16384, 2048] f32, weight/mask: [2048, 2048] f32, bias: [2048] f32.

The mask is a banded butterfly (|out - in| <= ~101) plus a dense first row
(output 0 reads all inputs) and dense first column (input 0 feeds all
outputs).  At 128-block granularity W^T is block-tridiagonal (46 blocks)
plus the global row/col, so the kernel only does ~1/5 of the dense work.

Design (measured ~62us on 8 cores, rel err 8.2e-3 vs the 2e-2 gate;
the previous fp16 pre-write design measured 72-74us):

  - data-parallel over tokens: 8 shards of 2048 tokens, one per NeuronCore
  - x in fp8 e4m3 (halves the dominant x load: 8.4 -> 4.2 MB/core) with
    HOST-SIDE ERROR-FEEDBACK quantization: greedily choose between the
    two neighboring fp8 values so the running o=0-weighted error
    E[n] = sum_i (q*w0q - x*w0)[n,i] stays bounded instead of growing
    sqrt(2048) - the dense o=0 output column is the only contraction
    long enough to breach the gate (plain RTN: 2.1e-2, feedback: 5e-3)
  - band W blocks in fp8 scaled by WSCALE=16 (pow2) to clear the e4m3
    subnormal knee; the whole device pipeline runs in the x16 domain and
    the host divides yt by 16 while unsharding.  Consecutive k-blocks
    pair into 256-contraction DoubleRow matmuls via 3D strided APs over
    the resident x/W tiles (no host re-layout): band PE 44 -> 33us
  - per output block bo (128 outputs), contract input blocks {bo-1, bo,
    bo+1}: one DR pair + one normal fp8 matmul per 512-col psum bank
  - the dense i=0 input row (rank-1) + bias are NOT psum pre-writes:
    ACT writes G(bo) = x0b*(16*w_in0[bo]) + 16*bias[bo] into SBUF,
    OUTSIDE the psum rotation, and the evacuation is a single DVE
    tensor_tensor add (ostage = ps + G).  Every group's first matmul is
    start=True (no virgin-bank priming), the psum chain is just
    matmul -> evac, and the PE runs 96% occupancy with zero ACT waits
    (the pre-write variant measured ~3us of PE stalls on ACT + priming)
  - x0b (x feature-row 0 replicated across partitions) is a host-built
    256KB input: a GpSimd partition_broadcast sourced from xall gained a
    tile-level dependency on every x batch and fired 6us late
  - the dense o=0 output column is 14 single-output (M=1, fp16 w0col)
    matmuls per token-slab, packed 4-wide via tile_position column
    groups (DoubleRow + tile_position does not compile - XBUS budget);
    the 4 partial rows are folded into y[:, 0] on the host
  - y is staged fp16 and stored fp16 ([out, tok], transposed) halving
    store traffic; the host un-transposes, unscales, and upcasts
  - DMA: ~600ns descgen per dma_start on the issuing sequencer, and a
    completion SEMAPHORE that trails the last byte by 2-4us (HBM write
    receipt under 8-core load) - the first matmul is sem-gated, so the
    first x batches lead the sync ring with nothing bulky ahead; W rides
    the same ring in 4 coarse batches (parking W descgen on the scalar
    ring delays the G computes, +4us; a SWDGE first batch, +6.6us)
  - the scalar ring carries only the three tiny early loads (w0col, x0b,
    bias/gwrow) and, at the tail, the last group's 4x512-col store
    pieces + psg stores, so the drain is not serialized behind the sync
    ring's store descgen
  - group order: bo=1 first (band carries the global row, needs only the
    first x blocks), 15 down to 3, bo=0 (needs all x blocks for the o=0
    column) second-to-last, ending on the cheap bo=2
  - remaining graded-time structure (core 0): ~6us NEFF init to first
    trigger + ~6us first-sem wait, ~41.5us PE span, ~5.5us evac/store/
    receipt drain, ~8.5us framework teardown barriers (fixed)

BGL_MODE / BGL_X8 / BGL_DR envs select dtype experiments (defaults:
fp16 weights-path, fp8 x, DoubleRow band on).
"""

import os

import numpy as np
import ml_dtypes

import concourse.bass as bass  # noqa: F401  (bass types via bacc)
import concourse.mybir as mybir
import concourse.tile as tile
from concourse import bacc
from concourse.bass_utils import run_bass_kernel_spmd


def _ensure_axon_hooks():
    """run_bass_kernel_spmd(trace=True) imports antenv.axon_hooks, which some
    images lack. Register the real libaxon-backed hook if available, else a
    no-op, so a BASS_TRACE=1 environment profiles instead of crashing."""
    import sys
    import types

    try:
        import antenv.axon_hooks  # noqa: F401
        return
    except ImportError:
        pass
    hook = None
    try:
        from trn_agent_boot.trn_boot import _ntff_profile_via_ctypes

        hook = _ntff_profile_via_ctypes("/opt/axon/libaxon_pjrt.so")
    except Exception:
        hook = None
    mod = types.ModuleType("antenv.axon_hooks")
    mod.get_axon_ntff_profile_hook = lambda: hook
    sys.modules["antenv.axon_hooks"] = mod


_ensure_axon_hooks()

MODE = os.environ.get("BGL_MODE", "fp16")
# x in fp8 e4m3 (halves the dominant x load DMA); host-side error-feedback
# quantization keeps the dense o=0 output column inside the error gate.
X8 = os.environ.get("BGL_X8", "1") == "1"
# DoubleRow: band W blocks in fp8 (scaled by WSCALE to clear the e4m3
# subnormal range; compensated by a 1/WSCALE at evacuation), consecutive
# k-blocks paired into one 256-contraction matmul -> ~1.4x band PE rate.
DR = X8 and os.environ.get("BGL_DR", "1") == "1"
WSCALE = 16.0

N_CORES = 8
TOK = 16384
F = 2048
P = 128
NB = F // P            # 16 feature blocks
NFREE = 512            # psum free dim (one bank of fp32)

F32 = mybir.dt.float32
BF16 = mybir.dt.bfloat16
FP16 = mybir.dt.float16

# most recent run's results (exec_time_ns etc.) for test harnesses
LAST_RESULTS = None


def _kset(bo):
    """Band input blocks contracted for output block bo (tridiagonal)."""
    return [bi for bi in (bo - 1, bo, bo + 1) if 0 <= bi < NB]


# Start at bo=1 (its band carries the global row and needs only the
# first x blocks), walk 15 down to 3 (each adds one new x block), bo=0
# (needs every x block for the o=0 column) second-to-last, and end on
# bo=2 (nothing new to load, 3 matmuls) for a cheap tail.  (Ending on
# bo=0 instead measured +3.3us: the psg hoisting and PE start slipped.)
BO_ORDER = [1] + list(range(15, 2, -1)) + [0, 2]


def _load_kset(bo):
    """x blocks whose tiles bo's group consumes (bo=0 also feeds the
    column-packed o=0 global reduction over every block)."""
    return list(range(NB)) if bo == 0 else _kset(bo)


def _wblocks():
    """(bo, bi) pairs needing a W^T block, in device compute order (so the
    packed slab can be streamed in exactly the order it is consumed)."""
    return [(bo, bi) for bo in BO_ORDER for bi in _kset(bo)]


_NC_CACHE = {}


def _build_nc(mode, tok_sh):
    """Build + compile the per-core Bass module (SPMD: same NEFF, 8 cores)."""
    if (mode, tok_sh, X8, DR) in _NC_CACHE:
        return _NC_CACHE[(mode, tok_sh, X8, DR)]
    wdt = {"fp16": FP16, "bf16": BF16}[mode]
    xdt = mybir.dt.float8e4 if X8 else wdt
    bdt = mybir.dt.float8e4 if DR else wdt  # band W slab dtype
    ns_count = tok_sh // NFREE
    blocks = _wblocks()
    bidx = {pair: i for i, pair in enumerate(blocks)}

    nc = bacc.Bacc("TRN2", target_bir_lowering=False, debug=False)

    # W^T blocks host-packed dense: slab j holds block (bo,bi)=blocks[j].
    # 3D [P, nblocks, P] so a DoubleRow matmul can take two adjacent slabs
    # as its two k-subtiles via one strided AP.
    w_dram = nc.dram_tensor("w", [P, len(blocks), P], bdt, kind="ExternalInput")
    # x^T partition-major: row p holds [block0 | block1 | ...] so any
    # column-range load is one DMA trigger with contiguous rows; 3D for the
    # same DoubleRow k-pair slicing (adjacent blocks at stride tok_sh).
    x_dram = nc.dram_tensor("x", [P, NB, tok_sh], xdt, kind="ExternalInput")
    # bias and the dense-input W^T row, merged into one load:
    # [:, :NB] = bias ([p, bo] -> bias[bo*128+p]); [:, NB:] = gwrow
    # ([p, bo] -> W^T[0, bo*128+p], cols 0,1 zeroed on the host since
    # blocks 0/1 carry the global row via the band)
    bg_dram = nc.dram_tensor("bg_pf", [P, 2 * NB], F32, kind="ExternalInput")
    # o=0 output column of W^T, blocked: column bi = W^T[bi*128:(bi+1)*128, 0]
    w0_dram = nc.dram_tensor("w0col", [P, NB], wdt, kind="ExternalInput")
    # x feature-row 0 of this shard, host-replicated across 128 partitions
    # (a 256KB load replaces a GpSimd partition_broadcast whose tile-level
    # dependency on xall serialized it behind every x batch)
    x0b_dram = nc.dram_tensor("x0b", [P, tok_sh], xdt, kind="ExternalInput")
    yt_dram = nc.dram_tensor("yt", [F, tok_sh], wdt, kind="ExternalOutput")
    # o=0 column partial sums (one row per tile_position column group);
    # the host folds these into y[:, 0] (unsharding the K-parallel split)
    psg_dram = nc.dram_tensor("psg", [4, tok_sh], F32, kind="ExternalOutput")

    with tile.TileContext(nc) as tc:
        with (
            tc.tile_pool(name="wpool", bufs=1) as wpool,
            tc.tile_pool(name="xpool", bufs=1) as xpool,
            tc.tile_pool(name="gpool", bufs=1) as gpool,
            tc.tile_pool(name="gtpool", bufs=4) as gtpool,
            tc.tile_pool(name="pspool", bufs=8, space="PSUM") as pspool,
            tc.tile_pool(name="opool", bufs=2) as opool,
        ):
            # the merged bias/gwrow scalars are the FIRST sync-ring trigger
            # (one 16KB dma): the first pre-write needs them ~8us in, and
            # the ACT ring would serialize them behind its table load
            # all three small early tensors ride the scalar ring: their
            # completion semaphores clear by ~10us while the sync ring is
            # still deep in x traffic
            x0b_sb = gpool.tile([P, tok_sh], xdt, tag="x0b")
            bg_sb = gpool.tile([P, 2 * NB], F32, tag="bg")
            w0_sb = gpool.tile([P, NB], wdt, tag="w0col")
            nc.scalar.dma_start(w0_sb[:], w0_dram[:, :])
            nc.scalar.dma_start(x0b_sb[:], x0b_dram[:, :])
            nc.scalar.dma_start(bg_sb[:], bg_dram[:, :])

            # x^T fully resident in one [128, NB, tok_sh] tile
            xall = xpool.tile([P, NB, tok_sh], xdt, tag="xall", name="xall")
            # resident packed W^T slab; block j = [:, j, :]
            wsb = wpool.tile([P, len(blocks), P], bdt, tag="w", name="w")

            # Issue loads in first-use order, but BATCHED: every dma_start
            # costs ~600ns of sequencer descriptor-gen time.  Singles up
            # front for a fast PE start, then progressively larger merged
            # ranges.  All bulk on the sync ring — a SWDGE (gpsimd) ring
            # for the first batch measured +6.6us, and parking W descgen on
            # the scalar ring delays the G computations.
            # x blocks in first-use order: 14,15,13,12,...,2,1,0 (descending
            # after the first two), so merged ranges stay contiguous.
            # (Leading with the [0,1] DoubleRow pair + [2] instead measured
            # +0.9us: the extra trigger slot delayed the first semaphore.)
            xbatches = [[0], [1, 2], [14, 15], [13], [12], [11, 10], [9, 8],
                        [7, 6], [5, 4], [3]]
            # coarse W j-ranges in first-use order (bo groups 1,15,14 |
            # 13,12,11 | 10..7 | rest), interleaved after the x batch that
            # feeds the same groups
            wseq = {1: (0, 8), 3: (8, 17), 5: (17, 29), 7: (29, len(blocks))}
            # flatten the 3D slices back to one contiguous run per
            # partition: sliced 3D APs otherwise emit one descriptor per
            # (partition, block) and halve early-load efficiency
            for k, bis in enumerate(xbatches):
                lo, hi = min(bis), max(bis) + 1
                nc.sync.dma_start(
                    xall[:, lo:hi, :].rearrange("p b t -> p (b t)"),
                    x_dram[:, lo:hi, :].rearrange("p b t -> p (b t)"),
                )
                if k in wseq:
                    jlo, jhi = wseq[k]
                    nc.sync.dma_start(
                        wsb[:, jlo:jhi, :].rearrange("p b w -> p (b w)"),
                        w_dram[:, jlo:jhi, :].rearrange("p b w -> p (b w)"),
                    )

            nw = 2 * NFREE  # 1024-col groups: 2 psum banks, halves the
            nw_count = tok_sh // nw  # fixed per-op overhead of evac ops

            # The global-input rank-1 term + bias live in SBUF "G" buffers,
            # computed by ACT OUTSIDE the psum rotation: every band group's
            # first matmul is start=True (no psum pre-writes, no virgin-bank
            # priming), and the evacuation is one DVE tensor_tensor add of
            # ps + G.  The PE never waits on ACT, and the psum chain is just
            # matmul -> evac.  Everything stays in the xWSCALE domain; the
            # host divides yt by WSCALE after gathering.

            for gi, bo in enumerate(BO_ORDER):
                ostage = opool.tile([P, tok_sh], wdt, tag="o", bufs=16)
                # G(bo) = x0b * (WSCALE*w_in0[bo]) + WSCALE*bias[bo] on ACT,
                # into SBUF: for bo 0/1 the gwrow column is zero, so it
                # degenerates to the broadcast bias.  Outside the psum
                # rotation, so ACT is never on the matmul critical path.
                gt = gtpool.tile([P, tok_sh], wdt, tag="gt", bufs=4)
                nc.scalar.activation(
                    gt[:],
                    x0b_sb[:],
                    mybir.ActivationFunctionType.Identity,
                    bias=bg_sb[:, bo : bo + 1],
                    scale=bg_sb[:, NB + bo : NB + bo + 1],
                )
                for nsw in range(nw_count):
                    wsl = slice(nsw * nw, (nsw + 1) * nw)
                    ps2 = pspool.tile([P, nw], F32, tag="ps", bufs=3)
                    has_global = False  # global term rides G at the evac
                    ks = _kset(bo)
                    for h in range(2):
                        tsl = slice((2 * nsw + h) * NFREE, (2 * nsw + h + 1) * NFREE)
                        if DR:
                            # pair the first two k-blocks (adjacent slabs in
                            # both wsb and xall) into one 256-contraction
                            # DoubleRow matmul; a third block rides normal.
                            j0 = bidx[(bo, ks[0])]
                            ops = [("dr", j0, ks[0])]
                            if len(ks) == 3:
                                ops.append(("n", bidx[(bo, ks[2])], ks[2]))
                            for i, (kind, j, bi) in enumerate(ops):
                                if kind == "dr":
                                    nc.tensor.matmul(
                                        ps2[:, h * NFREE : (h + 1) * NFREE],
                                        wsb[:, j : j + 2, :],
                                        xall[:, bi : bi + 2, tsl.start : tsl.stop],
                                        start=(not has_global and i == 0),
                                        stop=(i == len(ops) - 1),
                                        perf_mode=mybir.MatmulPerfMode.DoubleRow,
                                        skip_group_check=True,
                                    )
                                else:
                                    nc.tensor.matmul(
                                        ps2[:, h * NFREE : (h + 1) * NFREE],
                                        wsb[:, j, :],
                                        xall[:, bi, tsl.start : tsl.stop],
                                        start=(not has_global and i == 0),
                                        stop=(i == len(ops) - 1),
                                        skip_group_check=True,
                                    )
                        else:
                            for i, bi in enumerate(ks):
                                j = bidx[(bo, bi)]
                                nc.tensor.matmul(
                                    ps2[:, h * NFREE : (h + 1) * NFREE],
                                    wsb[:, j, :],
                                    xall[:, bi, tsl.start : tsl.stop],
                                    start=(not has_global and i == 0),
                                    stop=(i == len(ks) - 1),
                                    skip_group_check=True,
                                )

                    # evacuate psum -> fp16 staging: one DVE tensor_tensor
                    # fusing the global/bias term: ostage = ps + G (both in
                    # the xWSCALE domain; host unscales).  The last group is
                    # split into 512-col pieces so its stores fire
                    # incrementally.
                    if gi == len(BO_ORDER) - 1:
                        for h in range(2):
                            hs = slice(wsl.start + h * NFREE, wsl.start + (h + 1) * NFREE)
                            nc.vector.tensor_add(
                                ostage[:, hs], ps2[:, h * NFREE : (h + 1) * NFREE],
                                gt[:, hs],
                            )
                    else:
                        nc.vector.tensor_add(ostage[:, wsl], ps2[:], gt[:, wsl])

                if bo == 0:
                    # o=0 global column: every block bi>=2 contributes a
                    # single-output (M=1) matmul. Pack them 4-wide into the
                    # PE array via tile_position column groups so four
                    # stream concurrently; the 4 partial rows are stored and
                    # folded into y[:, 0] on the host (unsharding the
                    # K-parallel reduction).
                    psg_stage = gpool.tile(
                        [P, tok_sh], F32, tag="psg_stage", name="psg_stage"
                    )
                    units = list(range(2, NB))
                    ngrp = 4
                    per_grp = [[] for _ in range(ngrp)]
                    for idx, u in enumerate(units):
                        per_grp[idx % ngrp].append(u)
                    order = []
                    for slot in range(max(len(g) for g in per_grp)):
                        for jg in range(ngrp):
                            if slot < len(per_grp[jg]):
                                order.append((jg, slot, per_grp[jg][slot]))
                    for ns in range(ns_count):
                        tsl = slice(ns * NFREE, (ns + 1) * NFREE)
                        psg = pspool.tile(
                            [P, NFREE], F32, tag="psg", bufs=2, name="psg"
                        )
                        for jg, slot, bi in order:
                            nc.tensor.matmul(
                                psg[32 * jg : 32 * jg + 1, :],
                                w0_sb[:, bi : bi + 1],
                                xall[:, bi, tsl.start : tsl.stop],
                                start=(slot == 0),
                                stop=(slot == len(per_grp[jg]) - 1),
                                tile_position=(0, 32 * jg),
                            )
                        # DMA cannot read PSUM: stage partitions 0..96 to
                        # SBUF (cost is column-driven); on ACT, which has no
                        # psum pre-writes anymore and sits idle by the time
                        # psg runs, keeping DVE free for the band evacs
                        nc.scalar.activation(
                            psg_stage[0:97, tsl],
                            psg[0:97, :],
                            mybir.ActivationFunctionType.Copy,
                        )

                if bo == 0:
                    # psg partial-row stores on the scalar ring: it is idle
                    # at the tail, and keeps these four tiny triggers out of
                    # the sync ring's store descgen stream
                    for jg in range(4):
                        nc.scalar.dma_start(
                            psg_dram[jg, :],
                            psg_stage[32 * jg : 32 * jg + 1, :],
                        )

                # stores ride the SYNC queue, issued after every load: the
                # per-queue FIFO gives loads absolute priority over stores
                # for HBM bandwidth; ostage is buffered per-group (bufs=16)
                # so held-back stores don't stall evacuation.  The last
                # group stores in 512-col pieces on the (idle at the tail)
                # scalar ring, each firing as its evac piece lands.
                if gi == len(BO_ORDER) - 1:
                    for qs in range(tok_sh // NFREE):
                        qsl = slice(qs * NFREE, (qs + 1) * NFREE)
                        nc.scalar.dma_start(
                            yt_dram[bo * P : (bo + 1) * P, qsl], ostage[:, qsl]
                        )
                else:
                    nc.sync.dma_start(
                        yt_dram[bo * P : (bo + 1) * P, :], ostage[:]
                    )



    nc.compile()
    _NC_CACHE[(mode, tok_sh, X8)] = nc
    return nc


def _quantize_x_fb(x, w0):
    """e4m3-quantize x with error feedback targeted at the dense o=0 output
    row: greedily pick between the two neighboring fp8 values so the running
    weighted error E[n] = sum_i (q - x)[n, i] * w0[i] stays bounded (~max
    step * |w0|) instead of growing sqrt(F).  The o=0 column is the only
    2048-term contraction; its rms error would otherwise just exceed the
    2e-2 gate.  Band outputs (~200 terms) see at most one-step per-value
    error and stay 4x under the gate.  w0 must be the exact dequantized
    weight row the device contracts against (fp16 here)."""
    f8 = ml_dtypes.float8_e4m3
    x = x.astype(np.float32)
    n, f = x.shape
    xq = np.empty((n, f), dtype=f8)
    E = np.zeros(n, dtype=np.float32)
    w0 = w0.astype(np.float32)
    for i in range(f):
        v = x[:, i]
        q1 = v.astype(f8)
        r1 = q1.astype(np.float32) - v
        # the other-neighbor candidate: nudge past the rounding midpoint
        q2 = (v - r1 * np.float32(1.001)).astype(f8)
        r2 = q2.astype(np.float32) - v
        c1 = np.abs(E + r1 * w0[i])
        c2 = np.abs(E + r2 * w0[i])
        pick2 = c2 < c1
        xq[:, i] = np.where(pick2, q2, q1)
        E += np.where(pick2, r2, r1) * w0[i]
    return xq


def _prep_inputs(x, mask, weight, bias, mode, tok_sh):
    """Host-side layout prep -> per-core input maps."""
    npdt = {"fp16": np.float16, "bf16": ml_dtypes.bfloat16}[mode]
    n_sh = x.shape[0] // tok_sh

    w = mask.astype(np.float32) * weight.astype(np.float32)
    wtr = np.ascontiguousarray(w.T)  # [in, out]

    # pack the needed W^T blocks into a dense [128, nblocks, 128] slab
    blocks = _wblocks()
    packed = np.empty((P, len(blocks) * P), dtype=np.float32)
    for j, (bo, bi) in enumerate(blocks):
        packed[:, j * P : (j + 1) * P] = wtr[
            bi * P : (bi + 1) * P, bo * P : (bo + 1) * P
        ]
    if DR:
        # fp8 band: scale by WSCALE (pow2) so values clear the e4m3
        # subnormal knee; the evacuation multiplies psum by 1/WSCALE
        w_pk = (packed * WSCALE).astype(ml_dtypes.float8_e4m3)
    else:
        w_pk = packed.astype(npdt)
    w_pk = w_pk.reshape(P, len(blocks), P)

    # o=0 output column of W^T, blocked [128, NB] (fp16 even under DR:
    # mixed fp8-moving x fp16-stationary runs at full rate)
    w0col = np.ascontiguousarray(wtr[:, 0].reshape(NB, P).T).astype(npdt)

    bias_pf = bias.astype(np.float32).reshape(NB, P).T.copy()

    # dense-input row of W^T, blocked [128, NB]; zero the columns whose
    # band blocks already carry the global row (input block 0 in bo=0,1)
    gwrow = wtr[0, :].astype(np.float32).reshape(NB, P).T.copy()
    gwrow[:, 0] = 0.0
    gwrow[:, 1] = 0.0
    if DR:
        # the whole device computation lives in the xWSCALE domain (band W
        # is fp8 * WSCALE); G = x0b*(WSCALE*gwrow) + WSCALE*bias matches,
        # and the host divides yt by WSCALE after gathering
        gwrow *= WSCALE
        bias_pf *= WSCALE
    bg_pf = np.ascontiguousarray(np.concatenate([bias_pf, gwrow], axis=1))

    # per-core transposed x shards, partition-major: [core, 128, NB, tok]
    if X8:
        # feedback target = the exact dequantized per-feature weights the
        # device contracts against for output 0: fp8 band blocks (bo=0)
        # cover inputs 0..255, the fp16 w0col covers the rest
        weff = wtr[:, 0].astype(np.float16).astype(np.float32)
        if DR:
            for bi in (0, 1):
                j = blocks.index((0, bi))
                weff[bi * P : (bi + 1) * P] = (
                    w_pk[:, j, 0].astype(np.float32) / WSCALE
                )
        xq = _quantize_x_fb(x, weff)
        xs = xq.reshape(n_sh, tok_sh, F).transpose(0, 2, 1)
        x_h = np.ascontiguousarray(xs).reshape(n_sh, NB, P, tok_sh)
    else:
        xs = x.reshape(n_sh, tok_sh, F).transpose(0, 2, 1)
        x_h = np.ascontiguousarray(xs).astype(npdt).reshape(n_sh, NB, P, tok_sh)
    x_pm = np.ascontiguousarray(x_h.transpose(0, 2, 1, 3)).reshape(
        n_sh, P, NB, tok_sh
    )

    in_maps = []
    for c in range(n_sh):
        # x feature-row 0 replicated across partitions (device reads it as
        # the rank-1 dense-input term in the psum pre-writes)
        x0b = np.ascontiguousarray(
            np.broadcast_to(x_pm[c][0:1, 0, :], (P, tok_sh))
        )
        in_maps.append(
            {
                "bg_pf": bg_pf,
                "w": w_pk,
                "w0col": w0col,
                "x": x_pm[c],
                "x0b": x0b,
            }
        )
    return in_maps


def kernel(x, mask, weight, bias):
    global LAST_RESULTS
    x = np.asarray(x)
    tok, f = x.shape
    assert (tok, f) == (TOK, F), (tok, f)
    tok_sh = tok // N_CORES

    nc = _build_nc(MODE, tok_sh)
    in_maps = _prep_inputs(
        np.asarray(x), np.asarray(mask), np.asarray(weight), np.asarray(bias),
        MODE, tok_sh,
    )
    res = run_bass_kernel_spmd(nc, in_maps, list(range(N_CORES)))
    LAST_RESULTS = res

    y = np.empty((tok, F), dtype=np.float32)
    inv_ws = (1.0 / WSCALE) if DR else 1.0
    for c in range(N_CORES):
        sl = slice(c * tok_sh, (c + 1) * tok_sh)
        y[sl, :] = res.results[c]["yt"].T.astype(np.float32) * inv_ws
        # unshard the K-parallel o=0 column reduction: fold the 4
        # column-group partial rows (unscaled domain) into y[:, 0]
        y[sl, 0] += res.results[c]["psg"].astype(np.float32).sum(axis=0)
    return y



# revision 71
# speedup vs baseline: 1.0403x; 1.0403x over previous
"""Trainium2 Bass kernel for ButterflyGlobalLinear:

    y = x @ (mask * weight)^T + bias

x: [16384, 2048] f32, weight/mask: [2048, 2048] f32, bias: [2048] f32.

The mask is a banded butterfly (|out - in| <= ~101) plus a dense first row
(output 0 reads all inputs) and dense first column (input 0 feeds all
outputs).  At 128-block granularity W^T is block-tridiagonal (46 blocks)
plus the global row/col, so the kernel only does ~1/5 of the dense work.

Design (measured ~62us on 8 cores, rel err 8.2e-3 vs the 2e-2 gate;
the previous fp16 pre-write design measured 72-74us):

  - data-parallel over tokens: 8 shards of 2048 tokens, one per NeuronCore
  - x in fp8 e4m3 (halves the dominant x load: 8.4 -> 4.2 MB/core) with
    HOST-SIDE ERROR-FEEDBACK quantization: greedily choose between the
    two neighboring fp8 values so the running o=0-weighted error
    E[n] = sum_i (q*w0q - x*w0)[n,i] stays bounded instead of growing
    sqrt(2048) - the dense o=0 output column is the only contraction
    long enough to breach the gate (plain RTN: 2.1e-2, feedback: 5e-3)
  - band W blocks in fp8 scaled by WSCALE=16 (pow2) to clear the e4m3
    subnormal knee; the whole device pipeline runs in the x16 domain and
    the host divides yt by 16 while unsharding.  Consecutive k-blocks
    pair into 256-contraction DoubleRow matmuls via 3D strided APs over
    the resident x/W tiles (no host re-layout): band PE 44 -> 33us
  - per output block bo (128 outputs), contract input blocks {bo-1, bo,
    bo+1}: one DR pair + one normal fp8 matmul per 512-col psum bank
  - the dense i=0 input row (rank-1) + bias are NOT psum pre-writes:
    ACT writes G(bo) = x0b*(16*w_in0[bo]) + 16*bias[bo] into SBUF,
    OUTSIDE the psum rotation, and the evacuation is a single DVE
    tensor_tensor add (ostage = ps + G).  Every group's first matmul is
    start=True (no virgin-bank priming), the psum chain is just
    matmul -> evac, and the PE runs 96% occupancy with zero ACT waits
    (the pre-write variant measured ~3us of PE stalls on ACT + priming)
  - x0b (x feature-row 0 replicated across partitions) is a host-built
    256KB input: a GpSimd partition_broadcast sourced from xall gained a
    tile-level dependency on every x batch and fired 6us late
  - the dense o=0 output column is 14 single-output (M=1, fp16 w0col)
    matmuls per token-slab, packed 4-wide via tile_position column
    groups (DoubleRow + tile_position does not compile - XBUS budget);
    the 4 partial rows are folded into y[:, 0] on the host
  - y is staged fp16 and stored fp16 ([out, tok], transposed) halving
    store traffic; the host un-transposes, unscales, and upcasts
  - DMA: ~600ns descgen per dma_start on the issuing sequencer, and a
    completion SEMAPHORE that trails the last byte by 2-4us (HBM write
    receipt under 8-core load) - the first matmul is sem-gated, so the
    first x batches lead the sync ring with nothing bulky ahead; W rides
    the same ring in 4 coarse batches (parking W descgen on the scalar
    ring delays the G computes, +4us; a SWDGE first batch, +6.6us)
  - the scalar ring carries only the three tiny early loads (w0col, x0b,
    bias/gwrow) and, at the tail, the last group's 4x512-col store
    pieces + psg stores, so the drain is not serialized behind the sync
    ring's store descgen
  - group order: bo=1 first (band carries the global row, needs only the
    first x blocks), 15 down to 3, bo=0 (needs all x blocks for the o=0
    column) second-to-last, ending on the cheap bo=2
  - remaining graded-time structure (core 0): ~6us NEFF init to first
    trigger + ~6us first-sem wait, ~41.5us PE span, ~5.5us evac/store/
    receipt drain, ~8.5us framework teardown barriers (fixed)

BGL_MODE / BGL_X8 / BGL_DR envs select dtype experiments (defaults:
fp16 weights-path, fp8 x, DoubleRow band on).
"""

import os

import numpy as np
import ml_dtypes

import concourse.bass as bass  # noqa: F401  (bass types via bacc)
import concourse.mybir as mybir
import concourse.tile as tile
from concourse import bacc
from concourse.bass_utils import run_bass_kernel_spmd


def _ensure_axon_hooks():
    """run_bass_kernel_spmd(trace=True) imports antenv.axon_hooks, which some
    images lack. Register the real libaxon-backed hook if available, else a
    no-op, so a BASS_TRACE=1 environment profiles instead of crashing."""
    import sys
    import types

    try:
        import antenv.axon_hooks  # noqa: F401
        return
    except ImportError:
        pass
    hook = None
    try:
        from trn_agent_boot.trn_boot import _ntff_profile_via_ctypes

        hook = _ntff_profile_via_ctypes("/opt/axon/libaxon_pjrt.so")
    except Exception:
        hook = None
    mod = types.ModuleType("antenv.axon_hooks")
    mod.get_axon_ntff_profile_hook = lambda: hook
    sys.modules["antenv.axon_hooks"] = mod


_ensure_axon_hooks()

MODE = os.environ.get("BGL_MODE", "fp16")
# x in fp8 e4m3 (halves the dominant x load DMA); host-side error-feedback
# quantization keeps the dense o=0 output column inside the error gate.
X8 = os.environ.get("BGL_X8", "1") == "1"
# DoubleRow: band W blocks in fp8 (scaled by WSCALE to clear the e4m3
# subnormal range; compensated by a 1/WSCALE at evacuation), consecutive
# k-blocks paired into one 256-contraction matmul -> ~1.4x band PE rate.
DR = X8 and os.environ.get("BGL_DR", "1") == "1"
WSCALE = 16.0

N_CORES = 8
TOK = 16384
F = 2048
P = 128
NB = F // P            # 16 feature blocks
NFREE = 512            # psum free dim (one bank of fp32)

F32 = mybir.dt.float32
BF16 = mybir.dt.bfloat16
FP16 = mybir.dt.float16

# most recent run's results (exec_time_ns etc.) for test harnesses
LAST_RESULTS = None


def _kset(bo):
    """Band input blocks contracted for output block bo (tridiagonal)."""
    return [bi for bi in (bo - 1, bo, bo + 1) if 0 <= bi < NB]


# Start at bo=1 (its band carries the global row and needs only the
# first x blocks), walk 15 down to 3 (each adds one new x block), bo=0
# (needs every x block for the o=0 column) second-to-last, and end on
# bo=2 (nothing new to load, 3 matmuls) for a cheap tail.  (Ending on
# bo=0 instead measured +3.3us: the psg hoisting and PE start slipped.)
BO_ORDER = [1] + list(range(15, 2, -1)) + [0, 2]


def _load_kset(bo):
    """x blocks whose tiles bo's group consumes (bo=0 also feeds the
    column-packed o=0 global reduction over every block)."""
    return list(range(NB)) if bo == 0 else _kset(bo)


def _wblocks():
    """(bo, bi) pairs needing a W^T block, in device compute order (so the
    packed slab can be streamed in exactly the order it is consumed)."""
    return [(bo, bi) for bo in BO_ORDER for bi in _kset(bo)]


_NC_CACHE = {}


def _build_nc(mode, tok_sh):
    """Build + compile the per-core Bass module (SPMD: same NEFF, 8 cores)."""
    if (mode, tok_sh, X8, DR) in _NC_CACHE:
        return _NC_CACHE[(mode, tok_sh, X8, DR)]
    wdt = {"fp16": FP16, "bf16": BF16}[mode]
    xdt = mybir.dt.float8e4 if X8 else wdt
    bdt = mybir.dt.float8e4 if DR else wdt  # band W slab dtype
    ns_count = tok_sh // NFREE
    blocks = _wblocks()
    bidx = {pair: i for i, pair in enumerate(blocks)}

    nc = bacc.Bacc("TRN2", target_bir_lowering=False, debug=False)

    # W^T blocks host-packed dense: slab j holds block (bo,bi)=blocks[j].
    # 3D [P, nblocks, P] so a DoubleRow matmul can take two adjacent slabs
    # as its two k-subtiles via one strided AP.
    w_dram = nc.dram_tensor("w", [P, len(blocks), P], bdt, kind="ExternalInput")
    # x^T partition-major: row p holds [block0 | block1 | ...] so any
    # column-range load is one DMA trigger with contiguous rows; 3D for the
    # same DoubleRow k-pair slicing (adjacent blocks at stride tok_sh).
    x_dram = nc.dram_tensor("x", [P, NB, tok_sh], xdt, kind="ExternalInput")
    # bias and the dense-input W^T row, merged into one load:
    # [:, :NB] = bias ([p, bo] -> bias[bo*128+p]); [:, NB:] = gwrow
    # ([p, bo] -> W^T[0, bo*128+p], cols 0,1 zeroed on the host since
    # blocks 0/1 carry the global row via the band)
    bg_dram = nc.dram_tensor("bg_pf", [P, 2 * NB], F32, kind="ExternalInput")
    # o=0 output column of W^T, blocked: column bi = W^T[bi*128:(bi+1)*128, 0]
    w0_dram = nc.dram_tensor("w0col", [P, NB], wdt, kind="ExternalInput")
    # x feature-row 0 of this shard, host-replicated across 128 partitions
    # (a 256KB load replaces a GpSimd partition_broadcast whose tile-level
    # dependency on xall serialized it behind every x batch)
    x0b_dram = nc.dram_tensor("x0b", [P, tok_sh], xdt, kind="ExternalInput")
    yt_dram = nc.dram_tensor("yt", [F, tok_sh], wdt, kind="ExternalOutput")
    # o=0 column partial sums (one row per tile_position column group);
    # the host folds these into y[:, 0] (unsharding the K-parallel split)
    psg_dram = nc.dram_tensor("psg", [4, tok_sh], F32, kind="ExternalOutput")

    with tile.TileContext(nc) as tc:
        with (
            tc.tile_pool(name="wpool", bufs=1) as wpool,
            tc.tile_pool(name="xpool", bufs=1) as xpool,
            tc.tile_pool(name="gpool", bufs=1) as gpool,
            tc.tile_pool(name="gtpool", bufs=4) as gtpool,
            tc.tile_pool(name="pspool", bufs=8, space="PSUM") as pspool,
            tc.tile_pool(name="opool", bufs=2) as opool,
        ):
            # the merged bias/gwrow scalars are the FIRST sync-ring trigger
            # (one 16KB dma): the first pre-write needs them ~8us in, and
            # the ACT ring would serialize them behind its table load
            # the small early tensors ride the scalar ring: their
            # completion semaphores clear by ~10us while the sync ring is
            # still deep in x traffic.  x0b (0.26MB, needed by the first G
            # compute only at ~15us) moves to the sync ring behind the
            # first x batches, out of the first-matmul gate's HBM window.
            x0b_sb = gpool.tile([P, tok_sh], xdt, tag="x0b")
            bg_sb = gpool.tile([P, 2 * NB], F32, tag="bg")
            w0_sb = gpool.tile([P, NB], wdt, tag="w0col")
            nc.scalar.dma_start(w0_sb[:], w0_dram[:, :])
            nc.scalar.dma_start(bg_sb[:], bg_dram[:, :])

            # x^T fully resident in one [128, NB, tok_sh] tile
            xall = xpool.tile([P, NB, tok_sh], xdt, tag="xall", name="xall")
            # resident packed W^T slab; block j = [:, j, :]
            wsb = wpool.tile([P, len(blocks), P], bdt, tag="w", name="w")

            # Issue loads in first-use order, but BATCHED: every dma_start
            # costs ~600ns of sequencer descriptor-gen time.  Singles up
            # front for a fast PE start, then progressively larger merged
            # ranges.  All bulk on the sync ring — a SWDGE (gpsimd) ring
            # for the first batch measured +6.6us, and parking W descgen on
            # the scalar ring delays the G computations.
            # x blocks in first-use order: 14,15,13,12,...,2,1,0 (descending
            # after the first two), so merged ranges stay contiguous.
            # (Leading with the [0,1] DoubleRow pair + [2] instead measured
            # +0.9us: the extra trigger slot delayed the first semaphore.)
            xbatches = [[0], [1, 2], [14, 15], [13], [12], [11, 10], [9, 8],
                        [7, 6], [5, 4], [3]]
            # coarse W j-ranges in first-use order (bo groups 1,15,14 |
            # 13,12,11 | 10..7 | rest), interleaved after the x batch that
            # feeds the same groups
            wseq = {1: (0, 8), 3: (8, 17), 5: (17, 29), 7: (29, len(blocks))}
            # flatten the 3D slices back to one contiguous run per
            # partition: sliced 3D APs otherwise emit one descriptor per
            # (partition, block) and halve early-load efficiency
            for k, bis in enumerate(xbatches):
                lo, hi = min(bis), max(bis) + 1
                nc.sync.dma_start(
                    xall[:, lo:hi, :].rearrange("p b t -> p (b t)"),
                    x_dram[:, lo:hi, :].rearrange("p b t -> p (b t)"),
                )
                if k in wseq:
                    jlo, jhi = wseq[k]
                    nc.sync.dma_start(
                        wsb[:, jlo:jhi, :].rearrange("p b w -> p (b w)"),
                        w_dram[:, jlo:jhi, :].rearrange("p b w -> p (b w)"),
                    )
                if k == 1:
                    # x0b after the first-gate batches and bo=1/15/14's W:
                    # ready (~13.5us) well before the first G compute
                    nc.sync.dma_start(x0b_sb[:], x0b_dram[:, :])

            nw = 2 * NFREE  # 1024-col groups: 2 psum banks, halves the
            nw_count = tok_sh // nw  # fixed per-op overhead of evac ops

            # The global-input rank-1 term + bias live in SBUF "G" buffers,
            # computed by ACT OUTSIDE the psum rotation: every band group's
            # first matmul is start=True (no psum pre-writes, no virgin-bank
            # priming), and the evacuation is one DVE tensor_tensor add of
            # ps + G.  The PE never waits on ACT, and the psum chain is just
            # matmul -> evac.  Everything stays in the xWSCALE domain; the
            # host divides yt by WSCALE after gathering.

            for gi, bo in enumerate(BO_ORDER):
                ostage = opool.tile([P, tok_sh], wdt, tag="o", bufs=16)
                # G(bo) = x0b * (WSCALE*w_in0[bo]) + WSCALE*bias[bo] on ACT,
                # into SBUF: for bo 0/1 the gwrow column is zero, so it
                # degenerates to the broadcast bias.  Outside the psum
                # rotation, so ACT is never on the matmul critical path.
                gt = gtpool.tile([P, tok_sh], wdt, tag="gt", bufs=4)
                nc.scalar.activation(
                    gt[:],
                    x0b_sb[:],
                    mybir.ActivationFunctionType.Identity,
                    bias=bg_sb[:, bo : bo + 1],
                    scale=bg_sb[:, NB + bo : NB + bo + 1],
                )
                for nsw in range(nw_count):
                    wsl = slice(nsw * nw, (nsw + 1) * nw)
                    ps2 = pspool.tile([P, nw], F32, tag="ps", bufs=3)
                    has_global = False  # global term rides G at the evac
                    ks = _kset(bo)
                    for h in range(2):
                        tsl = slice((2 * nsw + h) * NFREE, (2 * nsw + h + 1) * NFREE)
                        if DR:
                            # pair the first two k-blocks (adjacent slabs in
                            # both wsb and xall) into one 256-contraction
                            # DoubleRow matmul; a third block rides normal.
                            j0 = bidx[(bo, ks[0])]
                            ops = [("dr", j0, ks[0])]
                            if len(ks) == 3:
                                ops.append(("n", bidx[(bo, ks[2])], ks[2]))
                            for i, (kind, j, bi) in enumerate(ops):
                                if kind == "dr":
                                    nc.tensor.matmul(
                                        ps2[:, h * NFREE : (h + 1) * NFREE],
                                        wsb[:, j : j + 2, :],
                                        xall[:, bi : bi + 2, tsl.start : tsl.stop],
                                        start=(not has_global and i == 0),
                                        stop=(i == len(ops) - 1),
                                        perf_mode=mybir.MatmulPerfMode.DoubleRow,
                                        skip_group_check=True,
                                    )
                                else:
                                    nc.tensor.matmul(
                                        ps2[:, h * NFREE : (h + 1) * NFREE],
                                        wsb[:, j, :],
                                        xall[:, bi, tsl.start : tsl.stop],
                                        start=(not has_global and i == 0),
                                        stop=(i == len(ops) - 1),
                                        skip_group_check=True,
                                    )
                        else:
                            for i, bi in enumerate(ks):
                                j = bidx[(bo, bi)]
                                nc.tensor.matmul(
                                    ps2[:, h * NFREE : (h + 1) * NFREE],
                                    wsb[:, j, :],
                                    xall[:, bi, tsl.start : tsl.stop],
                                    start=(not has_global and i == 0),
                                    stop=(i == len(ks) - 1),
                                    skip_group_check=True,
                                )

                    # evacuate psum -> fp16 staging: one DVE tensor_tensor
                    # fusing the global/bias term: ostage = ps + G (both in
                    # the xWSCALE domain; host unscales).  The last group is
                    # split into 512-col pieces so its stores fire
                    # incrementally.
                    if gi == len(BO_ORDER) - 1:
                        for h in range(2):
                            hs = slice(wsl.start + h * NFREE, wsl.start + (h + 1) * NFREE)
                            nc.vector.tensor_add(
                                ostage[:, hs], ps2[:, h * NFREE : (h + 1) * NFREE],
                                gt[:, hs],
                            )
                    else:
                        nc.vector.tensor_add(ostage[:, wsl], ps2[:], gt[:, wsl])

                if bo == 0:
                    # o=0 global column: every block bi>=2 contributes a
                    # single-output (M=1) matmul. Pack them 4-wide into the
                    # PE array via tile_position column groups so four
                    # stream concurrently; the 4 partial rows are stored and
                    # folded into y[:, 0] on the host (unsharding the
                    # K-parallel reduction).
                    psg_stage = gpool.tile(
                        [P, tok_sh], F32, tag="psg_stage", name="psg_stage"
                    )
                    units = list(range(2, NB))
                    ngrp = 4
                    per_grp = [[] for _ in range(ngrp)]
                    for idx, u in enumerate(units):
                        per_grp[idx % ngrp].append(u)
                    order = []
                    for slot in range(max(len(g) for g in per_grp)):
                        for jg in range(ngrp):
                            if slot < len(per_grp[jg]):
                                order.append((jg, slot, per_grp[jg][slot]))
                    for ns in range(ns_count):
                        tsl = slice(ns * NFREE, (ns + 1) * NFREE)
                        psg = pspool.tile(
                            [P, NFREE], F32, tag="psg", bufs=2, name="psg"
                        )
                        for jg, slot, bi in order:
                            nc.tensor.matmul(
                                psg[32 * jg : 32 * jg + 1, :],
                                w0_sb[:, bi : bi + 1],
                                xall[:, bi, tsl.start : tsl.stop],
                                start=(slot == 0),
                                stop=(slot == len(per_grp[jg]) - 1),
                                tile_position=(0, 32 * jg),
                            )
                        # DMA cannot read PSUM: stage partitions 0..96 to
                        # SBUF (cost is column-driven); on ACT, which has no
                        # psum pre-writes anymore and sits idle by the time
                        # psg runs, keeping DVE free for the band evacs
                        nc.scalar.activation(
                            psg_stage[0:97, tsl],
                            psg[0:97, :],
                            mybir.ActivationFunctionType.Copy,
                        )

                if bo == 0:
                    # psg partial-row stores on the scalar ring: it is idle
                    # at the tail, and keeps these four tiny triggers out of
                    # the sync ring's store descgen stream
                    for jg in range(4):
                        nc.scalar.dma_start(
                            psg_dram[jg, :],
                            psg_stage[32 * jg : 32 * jg + 1, :],
                        )

                # stores ride the SYNC queue, issued after every load: the
                # per-queue FIFO gives loads absolute priority over stores
                # for HBM bandwidth; ostage is buffered per-group (bufs=16)
                # so held-back stores don't stall evacuation.  The last
                # group stores in 512-col pieces on the (idle at the tail)
                # scalar ring, each firing as its evac piece lands.
                if gi == len(BO_ORDER) - 1:
                    for qs in range(tok_sh // NFREE):
                        qsl = slice(qs * NFREE, (qs + 1) * NFREE)
                        nc.scalar.dma_start(
                            yt_dram[bo * P : (bo + 1) * P, qsl], ostage[:, qsl]
                        )
                else:
                    nc.sync.dma_start(
                        yt_dram[bo * P : (bo + 1) * P, :], ostage[:]
                    )



    nc.compile()
    _NC_CACHE[(mode, tok_sh, X8)] = nc
    return nc


def _quantize_x_fb(x, w0):
    """e4m3-quantize x with error feedback targeted at the dense o=0 output
    row: greedily pick between the two neighboring fp8 values so the running
    weighted error E[n] = sum_i (q - x)[n, i] * w0[i] stays bounded (~max
    step * |w0|) instead of growing sqrt(F).  The o=0 column is the only
    2048-term contraction; its rms error would otherwise just exceed the
    2e-2 gate.  Band outputs (~200 terms) see at most one-step per-value
    error and stay 4x under the gate.  w0 must be the exact dequantized
    weight row the device contracts against (fp16 here)."""
    f8 = ml_dtypes.float8_e4m3
    x = x.astype(np.float32)
    n, f = x.shape
    xq = np.empty((n, f), dtype=f8)
    E = np.zeros(n, dtype=np.float32)
    w0 = w0.astype(np.float32)
    for i in range(f):
        v = x[:, i]
        q1 = v.astype(f8)
        r1 = q1.astype(np.float32) - v
        # the other-neighbor candidate: nudge past the rounding midpoint
        q2 = (v - r1 * np.float32(1.001)).astype(f8)
        r2 = q2.astype(np.float32) - v
        c1 = np.abs(E + r1 * w0[i])
        c2 = np.abs(E + r2 * w0[i])
        pick2 = c2 < c1
        xq[:, i] = np.where(pick2, q2, q1)
        E += np.where(pick2, r2, r1) * w0[i]
    return xq


def _prep_inputs(x, mask, weight, bias, mode, tok_sh):
    """Host-side layout prep -> per-core input maps."""
    npdt = {"fp16": np.float16, "bf16": ml_dtypes.bfloat16}[mode]
    n_sh = x.shape[0] // tok_sh

    w = mask.astype(np.float32) * weight.astype(np.float32)
    wtr = np.ascontiguousarray(w.T)  # [in, out]

    # pack the needed W^T blocks into a dense [128, nblocks, 128] slab
    blocks = _wblocks()
    packed = np.empty((P, len(blocks) * P), dtype=np.float32)
    for j, (bo, bi) in enumerate(blocks):
        packed[:, j * P : (j + 1) * P] = wtr[
            bi * P : (bi + 1) * P, bo * P : (bo + 1) * P
        ]
    if DR:
        # fp8 band: scale by WSCALE (pow2) so values clear the e4m3
        # subnormal knee; the evacuation multiplies psum by 1/WSCALE
        w_pk = (packed * WSCALE).astype(ml_dtypes.float8_e4m3)
    else:
        w_pk = packed.astype(npdt)
    w_pk = w_pk.reshape(P, len(blocks), P)

    # o=0 output column of W^T, blocked [128, NB] (fp16 even under DR:
    # mixed fp8-moving x fp16-stationary runs at full rate)
    w0col = np.ascontiguousarray(wtr[:, 0].reshape(NB, P).T).astype(npdt)

    bias_pf = bias.astype(np.float32).reshape(NB, P).T.copy()

    # dense-input row of W^T, blocked [128, NB]; zero the columns whose
    # band blocks already carry the global row (input block 0 in bo=0,1)
    gwrow = wtr[0, :].astype(np.float32).reshape(NB, P).T.copy()
    gwrow[:, 0] = 0.0
    gwrow[:, 1] = 0.0
    if DR:
        # the whole device computation lives in the xWSCALE domain (band W
        # is fp8 * WSCALE); G = x0b*(WSCALE*gwrow) + WSCALE*bias matches,
        # and the host divides yt by WSCALE after gathering
        gwrow *= WSCALE
        bias_pf *= WSCALE
    bg_pf = np.ascontiguousarray(np.concatenate([bias_pf, gwrow], axis=1))

    # per-core transposed x shards, partition-major: [core, 128, NB, tok]
    if X8:
        # feedback target = the exact dequantized per-feature weights the
        # device contracts against for output 0: fp8 band blocks (bo=0)
        # cover inputs 0..255, the fp16 w0col covers the rest
        weff = wtr[:, 0].astype(np.float16).astype(np.float32)
        if DR:
            for bi in (0, 1):
                j = blocks.index((0, bi))
                weff[bi * P : (bi + 1) * P] = (
                    w_pk[:, j, 0].astype(np.float32) / WSCALE
                )
        xq = _quantize_x_fb(x, weff)
        xs = xq.reshape(n_sh, tok_sh, F).transpose(0, 2, 1)
        x_h = np.ascontiguousarray(xs).reshape(n_sh, NB, P, tok_sh)
    else:
        xs = x.reshape(n_sh, tok_sh, F).transpose(0, 2, 1)
        x_h = np.ascontiguousarray(xs).astype(npdt).reshape(n_sh, NB, P, tok_sh)
    x_pm = np.ascontiguousarray(x_h.transpose(0, 2, 1, 3)).reshape(
        n_sh, P, NB, tok_sh
    )

    in_maps = []
    for c in range(n_sh):
        # x feature-row 0 replicated across partitions (device reads it as
        # the rank-1 dense-input term in the psum pre-writes)
        x0b = np.ascontiguousarray(
            np.broadcast_to(x_pm[c][0:1, 0, :], (P, tok_sh))
        )
        in_maps.append(
            {
                "bg_pf": bg_pf,
                "w": w_pk,
                "w0col": w0col,
                "x": x_pm[c],
                "x0b": x0b,
            }
        )
    return in_maps


def kernel(x, mask, weight, bias):
    global LAST_RESULTS
    x = np.asarray(x)
    tok, f = x.shape
    assert (tok, f) == (TOK, F), (tok, f)
    tok_sh = tok // N_CORES

    nc = _build_nc(MODE, tok_sh)
    in_maps = _prep_inputs(
        np.asarray(x), np.asarray(mask), np.asarray(weight), np.asarray(bias),
        MODE, tok_sh,
    )
    res = run_bass_kernel_spmd(nc, in_maps, list(range(N_CORES)))
    LAST_RESULTS = res

    y = np.empty((tok, F), dtype=np.float32)
    inv_ws = (1.0 / WSCALE) if DR else 1.0
    for c in range(N_CORES):
        sl = slice(c * tok_sh, (c + 1) * tok_sh)
        y[sl, :] = res.results[c]["yt"].T.astype(np.float32) * inv_ws
        # unshard the K-parallel o=0 column reduction: fold the 4
        # column-group partial rows (unscaled domain) into y[:, 0]
        y[sl, 0] += res.results[c]["psg"].astype(np.float32).sum(axis=0)
    return y

